# revision 6
# baseline (speedup 1.0000x reference)
"""Trainium2 Bass kernel: atrous (dilated) multi-head attention block.

Computation (per reference):
  x [2, 4096, 1024] --atrous regroup (dil=4)--> xr [8, 1024, 1024]
  q/k/v = xr @ W{q,k,v}.T + b;  16 heads, dh=64
  probs = softmax(q k^T / 8);  ctx = probs v
  atted = ctx @ Wf.T + bf;  final = LN(atted + x)
  returns (final, atted)

Sharding: B*dil == 8 == n_cores, so each NeuronCore takes one atrous group
[1024, 1024] — pure data parallel, zero collectives. The host performs the
strided regroup/scatter and pre-transposes / pre-scales operands so the
device kernel needs no on-chip transposes.

Per-core schedule (all matmuls contract over the partition dim):
  xT  [P, KC, L] bf16 : lhsT for V-proj, rhs for Q/K-proj.  All DRAM inputs
      are host-laid-out as [partition, chunk, free] so each of the 16
      partition-block DMAs reads fully contiguous HBM (16KB runs).
  qT/kT [P, L] bf16 per chunk: head-transposed; scoresT_h = kT_h^T @ qT_h
      (K=dh=64).  The two heads of a chunk run on complementary 64-row PE
      tiles (tile_position (0,0)/(64,0)) with their matmuls interleaved so
      the tensor engine can execute them concurrently.
  expT = exp(scoresT) on ScalarE (Wq pre-scaled by 1/8 on host).
  v_aug [L, H, 65] bf16 : v with an appended ones-column per head, so the
      ctx matmul (lhsT=v_aug, rhs=expT) also yields softmax denominators in
      psum row 64.  Normalization: DVE reciprocal on the denominator row +
      one SBUF->SBUF broadcast DMA (no DRAM bounce).
  The ctx matmuls of pair hc-1 are interleaved into the scores loop of pair
      hc at jc granularity: the PE streams 128-mode ctx work while ScalarE's
      exp stream catches up on the scores psum buffers, instead of stalling
      on the 2-buffer psum WAR.
  atted = ctxT^T @ WfT (K=D); +bias on DVE, residual+LN epilogue; outputs
      written bf16 (host upcasts), halving the output DMA burst.
"""

import os
import sys
from contextlib import ExitStack

for _p in ("/opt/trn_rl_repo",):
    if os.path.isdir(_p) and _p not in sys.path:
        sys.path.insert(0, _p)

import numpy as np
import ml_dtypes

import concourse.bass as bass
import concourse.mybir as mybir
from concourse.tile import TileContext
from concourse.bass_utils import run_bass_kernel_spmd

B, S, D = 2, 4096, 1024
DIL = 4
NCORES = 8
L = S // DIL  # 1024 rows per core
H, DH = 16, 64
P = 128
KC = D // P  # 8 contraction chunks
MT = D // P  # 8 output chunks
NT = 512  # matmul free-dim tile
NBLK = 16  # partition-block DMA splits (one per DMA engine)
PB = P // NBLK
EPS = 1e-5
SCALE = 1.0 / 8.0  # 1/sqrt(dh)

F32 = mybir.dt.float32
BF16 = mybir.dt.bfloat16
AL = mybir.AluOpType
AF = mybir.ActivationFunctionType
BF16_NP = ml_dtypes.bfloat16


def _split_excess_waits(nc: bass.Bass, max_waits: int = 1) -> None:
    """This neuronxcc's walrus rejects instructions carrying more than
    `max_waits` semaphore waits ("Too many sync wait commands").  Tile's
    kernel-tail drain (and occasionally a compute op) can exceed that.
    Move the excess waits onto same-engine no-ops inserted just before the
    instruction — the engine executes in order, so the happens-before
    relation is preserved exactly."""
    n = 0
    for fn in nc.m.functions:
        for blk in fn.blocks:
            insts = list(blk.instructions)
            out = []
            changed = False
            for inst in insts:
                si = inst.sync_info
                waits = list(si.on_wait) if (si is not None and si.on_wait) else []
                if len(waits) > max_waits:
                    changed = True
                    excess, keep = waits[:-max_waits], waits[-max_waits:]
                    for i in range(0, len(excess), max_waits):
                        nop = mybir.InstNoOp(name=f"waitsplit-{n}", ins=[], outs=[])
                        n += 1
                        nop.engine = inst.engine
                        nop.sync_info = mybir.SyncInfo(
                            on_wait=excess[i : i + max_waits], on_update=[]
                        )
                        nc.register_instruction(nop)
                        out.append(nop)
                    si.on_wait = keep
                out.append(inst)
            if changed:
                blk.instructions = out


def build_graph(apply_affine: bool = False) -> bass.Bass:
    nc = bass.Bass()
    xT_e = nc.declare_dram_parameter("xT", [P, KC, L], BF16, isOutput=False)
    xn_e = nc.declare_dram_parameter("xn", [P, MT, D], BF16, isOutput=False)
    wq_e = nc.declare_dram_parameter("wqT", [P, KC, D], BF16, isOutput=False)
    wk_e = nc.declare_dram_parameter("wkT", [P, KC, D], BF16, isOutput=False)
    wv_e = nc.declare_dram_parameter("wvT", [P, KC, D], BF16, isOutput=False)
    wf_e = nc.declare_dram_parameter("wfT", [P, KC, D], BF16, isOutput=False)
    bqc_e = nc.declare_dram_parameter("bqc", [P, MT], F32, isOutput=False)
    bkc_e = nc.declare_dram_parameter("bkc", [P, MT], F32, isOutput=False)
    bv_e = nc.declare_dram_parameter("bv", [D], F32, isOutput=False)
    bf_e = nc.declare_dram_parameter("bf", [D], F32, isOutput=False)
    gam_e = nc.declare_dram_parameter("gam", [D], F32, isOutput=False)
    bet_e = nc.declare_dram_parameter("bet", [D], F32, isOutput=False)
    out_e = nc.declare_dram_parameter("out", [2, L, D], BF16, isOutput=True)

    with TileContext(nc) as tc, ExitStack() as ctx:
        const = ctx.enter_context(tc.tile_pool(name="const", bufs=1))
        persist = ctx.enter_context(tc.tile_pool(name="persist", bufs=1))
        wpool = ctx.enter_context(tc.tile_pool(name="wpool", bufs=3))
        epool = ctx.enter_context(tc.tile_pool(name="epool", bufs=3))
        # PSUM: spool 2x[P,L] (4 banks) + cpool 2x[P,NT] (2) + mmps 2x[P,NT]
        # (2) = 8 banks exactly.
        mmps = ctx.enter_context(tc.tile_pool(name="mmps", bufs=2, space="PSUM"))
        cpool = ctx.enter_context(tc.tile_pool(name="cpool", bufs=2, space="PSUM"))
        spool = ctx.enter_context(tc.tile_pool(name="spool", bufs=2, space="PSUM"))
        npool = ctx.enter_context(tc.tile_pool(name="npool", bufs=2))
        dpool = ctx.enter_context(tc.tile_pool(name="dpool", bufs=2, space="DRAM"))
        opool = ctx.enter_context(tc.tile_pool(name="opool", bufs=2))
        sqp = ctx.enter_context(tc.tile_pool(name="sqp", bufs=1))
        stat = ctx.enter_context(tc.tile_pool(name="stat", bufs=4))

        def dma_blocks(dst, src):
            for i in range(NBLK):
                nc.sync.dma_start(
                    out=dst[i * PB : (i + 1) * PB], in_=src[i * PB : (i + 1) * PB]
                )

        # ---- whole-kernel inputs.  xT first (gates everything), then the
        # weights in consumption order.
        xT_sb = persist.tile([P, KC, L], BF16, tag="xT")
        dma_blocks(xT_sb, xT_e)
        bqc_sb = const.tile([P, MT], F32, tag="bqc")
        nc.sync.dma_start(out=bqc_sb[:], in_=bqc_e[:])
        bkc_sb = const.tile([P, MT], F32, tag="bkc")
        nc.sync.dma_start(out=bkc_sb[:], in_=bkc_e[:])
        bvb = const.tile([P, D], F32, tag="bvb")
        nc.sync.dma_start(out=bvb[:], in_=bv_e[None, :].to_broadcast((P, D)))
        bfb = const.tile([P, D], F32, tag="bfb")
        nc.sync.dma_start(out=bfb[:], in_=bf_e[None, :].to_broadcast((P, D)))
        if apply_affine:
            gmb = const.tile([P, D], F32, tag="gmb")
            nc.sync.dma_start(out=gmb[:], in_=gam_e[None, :].to_broadcast((P, D)))
            btb = const.tile([P, D], F32, tag="btb")
            nc.sync.dma_start(out=btb[:], in_=bet_e[None, :].to_broadcast((P, D)))
        epsb = const.tile([P, 1], F32, tag="epsb")
        nc.vector.memset(epsb[:], EPS)

        # per-chunk persistent arrays (separate tiles => fine-grained deps)
        qT = [persist.tile([P, L], BF16, tag=f"qT{m}", name=f"qT{m}") for m in range(MT)]
        kT = [persist.tile([P, L], BF16, tag=f"kT{m}", name=f"kT{m}") for m in range(MT)]
        vA = [persist.tile([P, H, DH + 1], BF16, tag=f"vA{m}", name=f"vA{m}") for m in range(KC)]
        cT = [persist.tile([P, L], BF16, tag=f"cT{m}", name=f"cT{m}") for m in range(KC)]
        for m in range(KC):
            nc.vector.memset(vA[m][:, :, DH : DH + 1], 1.0)

        def load_w(e, name):
            w = wpool.tile([P, KC, D], BF16, tag="w", name=name)
            dma_blocks(w, e)
            return w

        wv_sb = load_w(wv_e, "wv")
        wq_sb = load_w(wq_e, "wq")
        wk_sb = load_w(wk_e, "wk")

        # ---- emission helpers ------------------------------------------
        def v_group(m, t):
            """one V-projection accumulation group: psum[l_chunk, d_out]"""
            ps = mmps.tile([P, NT], F32, tag="mm", name="psv")
            for kc in range(KC):
                nc.tensor.matmul(
                    ps[:],
                    xT_sb[:, kc, m * P : (m + 1) * P],
                    wv_sb[:, kc, t * NT : (t + 1) * NT],
                    start=(kc == 0),
                    stop=(kc == KC - 1),
                )
            nc.vector.tensor_tensor(
                vA[m][:, t * 8 : (t + 1) * 8, 0:DH],
                ps[:].rearrange("p (h e) -> p h e", e=DH),
                bvb[:, t * NT : (t + 1) * NT].rearrange("p (h e) -> p h e", e=DH),
                AL.add,
            )

        def emit_qk(m):
            """Q/K projections for chunk m, head-transposed: psum = W^T @ xT;
            bias-add + bf16 cast on DVE (keeps ScalarE free for exp)."""
            for w_sb, bias_sb, dst in ((wq_sb, bqc_sb, qT), (wk_sb, bkc_sb, kT)):
                for t in range(2):
                    ps = mmps.tile([P, NT], F32, tag="mm", name="psqk")
                    for kc in range(KC):
                        nc.tensor.matmul(
                            ps[:],
                            w_sb[:, kc, m * P : (m + 1) * P],
                            xT_sb[:, kc, t * NT : (t + 1) * NT],
                            start=(kc == 0),
                            stop=(kc == KC - 1),
                        )
                    nc.vector.tensor_scalar(
                        dst[m][:, t * NT : (t + 1) * NT],
                        ps[:],
                        bias_sb[:, m : m + 1],
                        None,
                        AL.add,
                    )

        def scores_jc(hc, jc, eA, eB):
            """Scores+exp for both heads of chunk hc at key-chunk jc.  The
            even head occupies PE rows 0-63 (tile (0,0)), the odd head rows
            64-127 (tile (64,0)); alternating emission lets the engine run
            the two 64-row tiles concurrently."""
            psA = spool.tile([P, L], F32, tag="sc", name="scA")
            psB = spool.tile([P, L], F32, tag="sc", name="scB")
            for t in range(2):
                nc.tensor.matmul(
                    psA[:, t * NT : (t + 1) * NT],
                    kT[hc][0:DH, jc * P : (jc + 1) * P],
                    qT[hc][0:DH, t * NT : (t + 1) * NT],
                    start=True,
                    stop=True,
                    tile_position=(0, 0),
                )
                nc.tensor.matmul(
                    psB[:, t * NT : (t + 1) * NT],
                    kT[hc][DH:P, jc * P : (jc + 1) * P],
                    qT[hc][DH:P, t * NT : (t + 1) * NT],
                    start=True,
                    stop=True,
                    tile_position=(DH, 0),
                )
            nc.scalar.activation(eA[:, jc, :], psA[:], AF.Exp)
            nc.scalar.activation(eB[:, jc, :], psB[:], AF.Exp)

        def ctx_mm(h, t, eT, pc, jc):
            nc.tensor.matmul(
                pc[0 : DH + 1, :],
                vA[jc][:, h, :],
                eT[:, jc, t * NT : (t + 1) * NT],
                start=(jc == 0),
                stop=(jc == KC - 1),
            )

        def ctx_norm(h, t, pc):
            """psum -> normalized cT slice.  Denominators sit in psum row
            64; reciprocal directly on that row + one SBUF broadcast DMA."""
            hc, hh = h // 2, h % 2
            hp = hh * DH
            cn = npool.tile([DH + 1, NT], F32, tag="cn")
            nc.vector.tensor_copy(out=cn[:], in_=pc[0 : DH + 1, :])
            rT = npool.tile([1, NT], BF16, tag="rT")
            with nc.allow_low_precision(reason="1/den in bf16 is within tolerance"):
                nc.vector.reciprocal(rT[:], cn[DH : DH + 1, :])
            rd = dpool.tile([1, NT], BF16, tag="rd")
            nc.sync.dma_start(out=rd[:], in_=rT[:])
            rb = npool.tile([DH, NT], BF16, tag="rb")
            nc.sync.dma_start(out=rb[:], in_=rd[:].to_broadcast((DH, NT)))
            nc.vector.tensor_tensor(
                cT[hc][hp : hp + DH, t * NT : (t + 1) * NT],
                cn[0:DH, :],
                rb[:],
                AL.mult,
            )

        def emit_scores_pair(hc, ctx_work):
            """Fused: 8 jc steps of scores for pair hc, with `ctx_work` (a
            list of emission thunks, 4 groups x 8 jc from the previous pair)
            interleaved so the PE streams 128-mode ctx matmuls while
            ScalarE's exp stream drains the scores psums."""
            eA = epool.tile([P, KC, L], BF16, tag="eT", name=f"eT{2 * hc}")
            eB = epool.tile([P, KC, L], BF16, tag="eT", name=f"eT{2 * hc + 1}")
            nwork = len(ctx_work)
            done = 0
            for jc in range(KC):
                scores_jc(hc, jc, eA, eB)
                want = (jc + 1) * nwork // KC
                while done < want:
                    ctx_work[done]()
                    done += 1
            return eA, eB

        def make_ctx_work(pair, eA, eB):
            """4 accumulation groups (2 heads x 2 t-halves), each 8 jc
            matmuls + a normalization epilogue, as a flat thunk list."""
            work = []
            state = {}

            def mk(h, eT, t, jc):
                def run():
                    if jc == 0:
                        state[(h, t)] = cpool.tile([P, NT], F32, tag="cc", name=f"pc{h}_{t}")
                    ctx_mm(h, t, eT, state[(h, t)], jc)
                    if jc == KC - 1:
                        ctx_norm(h, t, state.pop((h, t)))
                return run

            for h, eT in ((2 * pair, eA), (2 * pair + 1, eB)):
                for t in range(2):
                    for jc in range(KC):
                        work.append(mk(h, eT, t, jc))
            return work

        # ---- emission ---------------------------------------------------
        v_group(0, 0)
        v_group(0, 1)
        v_group(1, 0)
        v_group(1, 1)
        emit_qk(0)
        vwork = [
            (lambda m=m, t=t: v_group(m, t)) for m in range(2, MT) for t in range(2)
        ]
        eA, eB = emit_scores_pair(0, vwork)

        xn_sb = wpool.tile([P, MT, D], BF16, tag="w", name="xn")  # reuses wv slot
        dma_blocks(xn_sb, xn_e)

        wf_sb = None
        for hc in range(1, KC):
            emit_qk(hc)
            if hc == KC - 1:
                wf_sb = load_w(wf_e, "wf")  # reuses wq slot (free after qk(7))
            work = make_ctx_work(hc - 1, eA, eB)
            eA, eB = emit_scores_pair(hc, work)

        # ---- last ctx pair interleaved with the output projection -------
        # out-proj m<4 needs only the t=0 halves of cT[7]; m>=4 needs t=1.
        def ctx_last(t):
            pcs = {}
            for h in (2 * (KC - 1), 2 * (KC - 1) + 1):
                eT = eA if h % 2 == 0 else eB
                pcs[h] = cpool.tile([P, NT], F32, tag="cc", name=f"pcL{h}")
                for jc in range(KC):
                    ctx_mm(h, t, eT, pcs[h], jc)
            for h in pcs:
                ctx_norm(h, t, pcs[h])

        def out_m(m):
            att = opool.tile([P, D], BF16, tag="att")
            for t in range(2):
                ps = mmps.tile([P, NT], F32, tag="mm", name="po")
                for kc in range(KC):
                    nc.tensor.matmul(
                        ps[:],
                        cT[kc][:, m * P : (m + 1) * P],
                        wf_sb[:, kc, t * NT : (t + 1) * NT],
                        start=(kc == 0),
                        stop=(kc == KC - 1),
                    )
                nc.vector.tensor_tensor(
                    att[:, t * NT : (t + 1) * NT],
                    ps[:],
                    bfb[:, t * NT : (t + 1) * NT],
                    AL.add,
                )
            nc.sync.dma_start(out=out_e[1, m * P : (m + 1) * P, :], in_=att[:])
            # LN epilogue
            res = opool.tile([P, D], F32, tag="res")
            ssum = stat.tile([P, 1], F32, tag="ss")
            nc.vector.scalar_tensor_tensor(
                res[:], att[:], 1.0, xn_sb[:, m, :], AL.mult, AL.add, accum_out=ssum[:]
            )
            sq = sqp.tile([P, D], BF16, tag="sq")
            sqs = stat.tile([P, 1], F32, tag="sqs")
            nc.scalar.activation(sq[:], res[:], AF.Square, accum_out=sqs[:])
            mu = stat.tile([P, 1], F32, tag="mu")
            nc.vector.tensor_scalar_mul(mu[:], ssum[:], 1.0 / D)
            msq = stat.tile([P, 1], F32, tag="msq")
            nc.vector.tensor_scalar(msq[:], mu[:], mu[:], None, AL.mult)
            var = stat.tile([P, 1], F32, tag="var")
            nc.vector.tensor_scalar(var[:], sqs[:], 1.0 / D, msq[:], AL.mult, AL.subtract)
            sd = stat.tile([P, 1], F32, tag="sd")
            nc.scalar.activation(sd[:], var[:], AF.Sqrt, bias=epsb[:])
            inv = stat.tile([P, 1], F32, tag="inv")
            nc.vector.reciprocal(inv[:], sd[:])
            # final LN output reuses the att tile (its atted DMA has drained
            # by the time the stats chain finishes)
            if apply_affine:
                nc.vector.tensor_scalar(res[:], res[:], mu[:], inv[:], AL.subtract, AL.mult)
                nc.vector.scalar_tensor_tensor(res[:], res[:], 1.0, gmb[:], AL.mult, AL.mult)
                nc.vector.tensor_tensor(att[:], res[:], btb[:], AL.add)
            else:
                nc.vector.tensor_scalar(att[:], res[:], mu[:], inv[:], AL.subtract, AL.mult)
            nc.sync.dma_start(out=out_e[0, m * P : (m + 1) * P, :], in_=att[:])

        ctx_last(0)
        for m in range(MT // 2):
            out_m(m)
        ctx_last(1)
        for m in range(MT // 2, MT):
            out_m(m)

    _split_excess_waits(nc)
    return nc


def prepare_in_maps(inputs):
    def chunk_pmajor(a):
        # [KC*P, N] -> [P, KC, N] so each partition's row is contiguous HBM
        kcp, n = a.shape
        return np.ascontiguousarray(
            a.reshape(KC, P, n).transpose(1, 0, 2)
        )

    x = np.asarray(inputs["x"], np.float32)
    xr = x.reshape(B, L, DIL, D).transpose(0, 2, 1, 3).reshape(NCORES, L, D)
    shared = {
        "wqT": chunk_pmajor((np.asarray(inputs["Wq"], np.float32).T * SCALE).astype(BF16_NP)),
        "wkT": chunk_pmajor(np.asarray(inputs["Wk"], np.float32).T.astype(BF16_NP)),
        "wvT": chunk_pmajor(np.asarray(inputs["Wv"], np.float32).T.astype(BF16_NP)),
        "wfT": chunk_pmajor(np.asarray(inputs["Wf"], np.float32).T.astype(BF16_NP)),
        "bqc": np.ascontiguousarray(
            (np.asarray(inputs["bq"], np.float32) * SCALE).reshape(MT, P).T
        ),
        "bkc": np.ascontiguousarray(
            np.asarray(inputs["bk"], np.float32).reshape(MT, P).T
        ),
        "bv": np.ascontiguousarray(inputs["bv"], dtype=np.float32),
        "bf": np.ascontiguousarray(inputs["bf"], dtype=np.float32),
        "gam": np.ascontiguousarray(inputs["gamma"], dtype=np.float32),
        "bet": np.ascontiguousarray(inputs["beta"], dtype=np.float32),
    }
    maps = []
    for c in range(NCORES):
        xs = np.ascontiguousarray(xr[c])
        m = dict(shared)
        m["xT"] = chunk_pmajor(np.ascontiguousarray(xs.T).astype(BF16_NP))
        m["xn"] = chunk_pmajor(xs.astype(BF16_NP))
        maps.append(m)
    return maps


def gather_outputs(results):
    outs = np.stack(
        [np.asarray(results[c]["out"]).astype(np.float32) for c in range(NCORES)]
    )
    final = outs[:, 0].reshape(B, DIL, L, D).transpose(0, 2, 1, 3).reshape(B, S, D)
    atted = outs[:, 1].reshape(B, DIL, L, D).transpose(0, 2, 1, 3).reshape(B, S, D)
    return np.ascontiguousarray(final), np.ascontiguousarray(atted)


_GRAPHS = {}


def get_graph(apply_affine=False):
    if apply_affine not in _GRAPHS:
        _GRAPHS[apply_affine] = build_graph(apply_affine)
    return _GRAPHS[apply_affine]


def run(inputs, trace=False, **kw):
    # gamma/beta are fixed to ones/zeros by the reference's setup_inputs;
    # only emit the affine LN ops if they are actually non-identity.
    apply_affine = not (
        np.all(np.asarray(inputs["gamma"]) == 1.0)
        and np.all(np.asarray(inputs["beta"]) == 0.0)
    )
    nc = get_graph(apply_affine)
    maps = prepare_in_maps(inputs)
    res = run_bass_kernel_spmd(nc, maps, core_ids=list(range(NCORES)), trace=trace, **kw)
    return gather_outputs(res.results), res


def kernel(**inputs):
    (final, atted), _ = run(inputs, trace=False)
    return final, atted


# revision 8
# speedup vs baseline: 1.1676x; 1.1676x over previous
"""Trainium2 Bass kernel: atrous (dilated) multi-head attention block.

Computation (per reference):
  x [2, 4096, 1024] --atrous regroup (dil=4)--> xr [8, 1024, 1024]
  q/k/v = xr @ W{q,k,v}.T + b;  16 heads, dh=64
  probs = softmax(q k^T / 8);  ctx = probs v
  atted = ctx @ Wf.T + bf;  final = LN(atted + x)
  returns (final, atted)

Sharding: B*dil == 8 == n_cores, so each NeuronCore takes one atrous group
[1024, 1024] — pure data parallel, zero collectives. The host performs the
strided regroup/scatter and pre-transposes / pre-scales operands so the
device kernel needs no on-chip transposes.

Per-core schedule (all matmuls contract over the partition dim):
  xT  [P, KC, L] bf16 : lhsT for V-proj, rhs for Q/K-proj.  All DRAM inputs
      are host-laid-out as [partition, chunk, free] so each of the 16
      partition-block DMAs reads fully contiguous HBM (16KB runs).
  qT/kT [P, L] bf16 per chunk: head-transposed; scoresT_h = kT_h^T @ qT_h
      (K=dh=64).  The two heads of a chunk run on complementary 64-row PE
      tiles (tile_position (0,0)/(64,0)) with their matmuls interleaved so
      the tensor engine can execute them concurrently.
  expT = exp(scoresT) on ScalarE (Wq pre-scaled by 1/8 on host).
  v_aug [L, H, 65] bf16 : v with an appended ones-column per head, so the
      ctx matmul (lhsT=v_aug, rhs=expT) also yields softmax denominators in
      psum row 64.  Normalization: DVE reciprocal on the denominator row +
      one SBUF->SBUF broadcast DMA (no DRAM bounce).
  The ctx matmuls of pair hc-1 are interleaved into the scores loop of pair
      hc at jc granularity: the PE streams 128-mode ctx work while ScalarE's
      exp stream catches up on the scores psum buffers, instead of stalling
      on the 2-buffer psum WAR.
  atted = ctxT^T @ WfT (K=D); +bias on DVE, residual+LN epilogue; outputs
      written bf16 (host upcasts), halving the output DMA burst.
"""

import os
import sys
from contextlib import ExitStack

for _p in ("/opt/trn_rl_repo",):
    if os.path.isdir(_p) and _p not in sys.path:
        sys.path.insert(0, _p)

import numpy as np
import ml_dtypes

import concourse.bass as bass
import concourse.mybir as mybir
from concourse.tile import TileContext
from concourse.bass_utils import run_bass_kernel_spmd

B, S, D = 2, 4096, 1024
DIL = 4
NCORES = 8
L = S // DIL  # 1024 rows per core
H, DH = 16, 64
P = 128
KC = D // P  # 8 contraction chunks
MT = D // P  # 8 output chunks
NT = 512  # matmul free-dim tile
NBLK = 16  # partition-block DMA splits (one per DMA engine)
PB = P // NBLK
EPS = 1e-5
SCALE = 1.0 / 8.0  # 1/sqrt(dh)

F32 = mybir.dt.float32
BF16 = mybir.dt.bfloat16
AL = mybir.AluOpType
AF = mybir.ActivationFunctionType
BF16_NP = ml_dtypes.bfloat16


def _split_excess_waits(nc: bass.Bass, max_waits: int = 1) -> None:
    """This neuronxcc's walrus rejects instructions carrying more than
    `max_waits` semaphore waits ("Too many sync wait commands").  Tile's
    kernel-tail drain (and occasionally a compute op) can exceed that.
    Move the excess waits onto same-engine no-ops inserted just before the
    instruction — the engine executes in order, so the happens-before
    relation is preserved exactly."""
    n = 0
    for fn in nc.m.functions:
        for blk in fn.blocks:
            insts = list(blk.instructions)
            out = []
            changed = False
            for inst in insts:
                si = inst.sync_info
                waits = list(si.on_wait) if (si is not None and si.on_wait) else []
                if len(waits) > max_waits:
                    changed = True
                    excess, keep = waits[:-max_waits], waits[-max_waits:]
                    for i in range(0, len(excess), max_waits):
                        nop = mybir.InstNoOp(name=f"waitsplit-{n}", ins=[], outs=[])
                        n += 1
                        nop.engine = inst.engine
                        nop.sync_info = mybir.SyncInfo(
                            on_wait=excess[i : i + max_waits], on_update=[]
                        )
                        nc.register_instruction(nop)
                        out.append(nop)
                    si.on_wait = keep
                out.append(inst)
            if changed:
                blk.instructions = out


def build_graph(apply_affine: bool = False) -> bass.Bass:
    nc = bass.Bass()
    xT_e = nc.declare_dram_parameter("xT", [P, KC, L], BF16, isOutput=False)
    xn_e = nc.declare_dram_parameter("xn", [P, MT, D], BF16, isOutput=False)
    wq_e = nc.declare_dram_parameter("wqT", [P, KC, D], BF16, isOutput=False)
    wk_e = nc.declare_dram_parameter("wkT", [P, KC, D], BF16, isOutput=False)
    wv_e = nc.declare_dram_parameter("wvT", [P, KC, D], BF16, isOutput=False)
    wf_e = nc.declare_dram_parameter("wfT", [P, KC, D], BF16, isOutput=False)
    bqc_e = nc.declare_dram_parameter("bqc", [P, MT], F32, isOutput=False)
    bkc_e = nc.declare_dram_parameter("bkc", [P, MT], F32, isOutput=False)
    bv_e = nc.declare_dram_parameter("bv", [D], F32, isOutput=False)
    bf_e = nc.declare_dram_parameter("bf", [D], F32, isOutput=False)
    gam_e = nc.declare_dram_parameter("gam", [D], F32, isOutput=False)
    bet_e = nc.declare_dram_parameter("bet", [D], F32, isOutput=False)
    out_e = nc.declare_dram_parameter("out", [2, L, D], BF16, isOutput=True)

    with TileContext(nc) as tc, ExitStack() as ctx:
        const = ctx.enter_context(tc.tile_pool(name="const", bufs=1))
        persist = ctx.enter_context(tc.tile_pool(name="persist", bufs=1))
        wpool = ctx.enter_context(tc.tile_pool(name="wpool", bufs=3))
        epool = ctx.enter_context(tc.tile_pool(name="epool", bufs=3))
        # PSUM: spool 2x[P,L] (4 banks) + cpool 2x[P,NT] (2) + mmps 2x[P,NT]
        # (2) = 8 banks exactly.
        mmps = ctx.enter_context(tc.tile_pool(name="mmps", bufs=2, space="PSUM"))
        cpool = ctx.enter_context(tc.tile_pool(name="cpool", bufs=2, space="PSUM"))
        spool = ctx.enter_context(tc.tile_pool(name="spool", bufs=2, space="PSUM"))
        npool = ctx.enter_context(tc.tile_pool(name="npool", bufs=2))
        dpool = ctx.enter_context(tc.tile_pool(name="dpool", bufs=2, space="DRAM"))
        opool = ctx.enter_context(tc.tile_pool(name="opool", bufs=2))
        sqp = ctx.enter_context(tc.tile_pool(name="sqp", bufs=1))
        stat = ctx.enter_context(tc.tile_pool(name="stat", bufs=4))

        def dma_blocks(dst, src):
            # one DMA per middle-dim chunk, spanning all 128 partitions:
            # [128, 1, 2-4KB] descriptors sustain ~190+ GB/s across queues
            # (8-partition blocks with long runs measured 3x slower).
            nchunk = dst.shape[1]
            for c in range(nchunk):
                nc.sync.dma_start(out=dst[:, c : c + 1], in_=src[:, c : c + 1])

        # ---- whole-kernel inputs.  xT first (gates everything), then the
        # weights in consumption order.
        xT_sb = persist.tile([P, KC, L], BF16, tag="xT")
        dma_blocks(xT_sb, xT_e)
        bqc_sb = const.tile([P, MT], F32, tag="bqc")
        nc.sync.dma_start(out=bqc_sb[:], in_=bqc_e[:])
        bkc_sb = const.tile([P, MT], F32, tag="bkc")
        nc.sync.dma_start(out=bkc_sb[:], in_=bkc_e[:])
        bvb = const.tile([P, D], F32, tag="bvb")
        nc.sync.dma_start(out=bvb[:], in_=bv_e[None, :].to_broadcast((P, D)))
        bfb = const.tile([P, D], F32, tag="bfb")
        nc.sync.dma_start(out=bfb[:], in_=bf_e[None, :].to_broadcast((P, D)))
        if apply_affine:
            gmb = const.tile([P, D], F32, tag="gmb")
            nc.sync.dma_start(out=gmb[:], in_=gam_e[None, :].to_broadcast((P, D)))
            btb = const.tile([P, D], F32, tag="btb")
            nc.sync.dma_start(out=btb[:], in_=bet_e[None, :].to_broadcast((P, D)))
        epsb = const.tile([P, 1], F32, tag="epsb")
        nc.vector.memset(epsb[:], EPS)

        # per-chunk persistent arrays (separate tiles => fine-grained deps)
        qT = [persist.tile([P, L], BF16, tag=f"qT{m}", name=f"qT{m}") for m in range(MT)]
        kT = [persist.tile([P, L], BF16, tag=f"kT{m}", name=f"kT{m}") for m in range(MT)]
        vA = [persist.tile([P, H, DH + 1], BF16, tag=f"vA{m}", name=f"vA{m}") for m in range(KC)]
        cT = [persist.tile([P, L], BF16, tag=f"cT{m}", name=f"cT{m}") for m in range(KC)]
        for m in range(KC):
            nc.vector.memset(vA[m][:, :, DH : DH + 1], 1.0)

        def load_w(e, name):
            w = wpool.tile([P, KC, D], BF16, tag="w", name=name)
            dma_blocks(w, e)
            return w

        wv_sb = load_w(wv_e, "wv")
        wq_sb = load_w(wq_e, "wq")
        wk_sb = load_w(wk_e, "wk")

        # ---- emission helpers ------------------------------------------
        def v_group(m, t):
            """one V-projection accumulation group: psum[l_chunk, d_out]"""
            ps = mmps.tile([P, NT], F32, tag="mm", name="psv")
            for kc in range(KC):
                nc.tensor.matmul(
                    ps[:],
                    xT_sb[:, kc, m * P : (m + 1) * P],
                    wv_sb[:, kc, t * NT : (t + 1) * NT],
                    start=(kc == 0),
                    stop=(kc == KC - 1),
                )
            nc.vector.tensor_tensor(
                vA[m][:, t * 8 : (t + 1) * 8, 0:DH],
                ps[:].rearrange("p (h e) -> p h e", e=DH),
                bvb[:, t * NT : (t + 1) * NT].rearrange("p (h e) -> p h e", e=DH),
                AL.add,
            )

        def emit_qk(m):
            """Q/K projections for chunk m, head-transposed: psum = W^T @ xT;
            bias-add + bf16 cast on DVE (keeps ScalarE free for exp)."""
            for w_sb, bias_sb, dst in ((wq_sb, bqc_sb, qT), (wk_sb, bkc_sb, kT)):
                for t in range(2):
                    ps = mmps.tile([P, NT], F32, tag="mm", name="psqk")
                    for kc in range(KC):
                        nc.tensor.matmul(
                            ps[:],
                            w_sb[:, kc, m * P : (m + 1) * P],
                            xT_sb[:, kc, t * NT : (t + 1) * NT],
                            start=(kc == 0),
                            stop=(kc == KC - 1),
                        )
                    nc.vector.tensor_scalar(
                        dst[m][:, t * NT : (t + 1) * NT],
                        ps[:],
                        bias_sb[:, m : m + 1],
                        None,
                        AL.add,
                    )

        def scores_jc(hc, jc, eA, eB):
            """Scores+exp for both heads of chunk hc at key-chunk jc.  The
            even head occupies PE rows 0-63 (tile (0,0)), the odd head rows
            64-127 (tile (64,0)); alternating emission lets the engine run
            the two 64-row tiles concurrently."""
            psA = spool.tile([P, L], F32, tag="sc", name="scA")
            psB = spool.tile([P, L], F32, tag="sc", name="scB")
            for t in range(2):
                nc.tensor.matmul(
                    psA[:, t * NT : (t + 1) * NT],
                    kT[hc][0:DH, jc * P : (jc + 1) * P],
                    qT[hc][0:DH, t * NT : (t + 1) * NT],
                    start=True,
                    stop=True,
                    tile_position=(0, 0),
                )
                nc.tensor.matmul(
                    psB[:, t * NT : (t + 1) * NT],
                    kT[hc][DH:P, jc * P : (jc + 1) * P],
                    qT[hc][DH:P, t * NT : (t + 1) * NT],
                    start=True,
                    stop=True,
                    tile_position=(DH, 0),
                )
            nc.scalar.activation(eA[:, jc, :], psA[:], AF.Exp)
            nc.scalar.activation(eB[:, jc, :], psB[:], AF.Exp)

        def ctx_mm(h, t, eT, pc, jc):
            nc.tensor.matmul(
                pc[0 : DH + 1, :],
                vA[jc][:, h, :],
                eT[:, jc, t * NT : (t + 1) * NT],
                start=(jc == 0),
                stop=(jc == KC - 1),
            )

        def ctx_norm(h, t, pc):
            """psum -> normalized cT slice.  Denominators sit in psum row
            64; reciprocal directly on that row + one SBUF broadcast DMA."""
            hc, hh = h // 2, h % 2
            hp = hh * DH
            cn = npool.tile([DH + 1, NT], F32, tag="cn")
            nc.vector.tensor_copy(out=cn[:], in_=pc[0 : DH + 1, :])
            # broadcast the raw denominator row to 64 partitions via a DRAM
            # bounce, THEN reciprocal on the [64, 512] tile — a [1, 512]
            # reciprocal runs on a single DVE lane (~2.7us); this way all
            # partitions work (~0.5us).
            rd = dpool.tile([1, NT], F32, tag="rd")
            nc.sync.dma_start(out=rd[:], in_=cn[DH : DH + 1, :])
            rb = npool.tile([DH, NT], F32, tag="rb")
            nc.sync.dma_start(out=rb[:], in_=rd[:].to_broadcast((DH, NT)))
            nc.vector.reciprocal(rb[:], rb[:])
            nc.vector.tensor_tensor(
                cT[hc][hp : hp + DH, t * NT : (t + 1) * NT],
                cn[0:DH, :],
                rb[:],
                AL.mult,
            )

        def emit_scores_pair(hc, ctx_work):
            """Fused: 8 jc steps of scores for pair hc, with `ctx_work` (a
            list of emission thunks, 4 groups x 8 jc from the previous pair)
            interleaved so the PE streams 128-mode ctx matmuls while
            ScalarE's exp stream drains the scores psums."""
            eA = epool.tile([P, KC, L], BF16, tag="eT", name=f"eT{2 * hc}")
            eB = epool.tile([P, KC, L], BF16, tag="eT", name=f"eT{2 * hc + 1}")
            nwork = len(ctx_work)
            done = 0
            for jc in range(KC):
                scores_jc(hc, jc, eA, eB)
                want = (jc + 1) * nwork // KC
                while done < want:
                    ctx_work[done]()
                    done += 1
            return eA, eB

        def make_ctx_work(pair, eA, eB):
            """4 accumulation groups (2 heads x 2 t-halves), each 8 jc
            matmuls + a normalization epilogue, as a flat thunk list."""
            work = []
            state = {}

            def mk(h, eT, t, jc):
                def run():
                    if jc == 0:
                        state[(h, t)] = cpool.tile([P, NT], F32, tag="cc", name=f"pc{h}_{t}")
                    ctx_mm(h, t, eT, state[(h, t)], jc)
                    if jc == KC - 1:
                        ctx_norm(h, t, state.pop((h, t)))
                return run

            for h, eT in ((2 * pair, eA), (2 * pair + 1, eB)):
                for t in range(2):
                    for jc in range(KC):
                        work.append(mk(h, eT, t, jc))
            return work

        # ---- emission ---------------------------------------------------
        v_group(0, 0)
        v_group(0, 1)
        v_group(1, 0)
        v_group(1, 1)
        emit_qk(0)
        vwork = [
            (lambda m=m, t=t: v_group(m, t)) for m in range(2, MT) for t in range(2)
        ]
        eA, eB = emit_scores_pair(0, vwork)

        xn_sb = wpool.tile([P, MT, D], BF16, tag="w", name="xn")  # reuses wv slot
        dma_blocks(xn_sb, xn_e)

        wf_sb = None
        for hc in range(1, KC):
            emit_qk(hc)
            if hc == KC - 1:
                wf_sb = load_w(wf_e, "wf")  # reuses wq slot (free after qk(7))
            work = make_ctx_work(hc - 1, eA, eB)
            eA, eB = emit_scores_pair(hc, work)

        # ---- last ctx pair interleaved with the output projection -------
        # out-proj m<4 needs only the t=0 halves of cT[7]; m>=4 needs t=1.
        def ctx_last(t):
            pcs = {}
            for h in (2 * (KC - 1), 2 * (KC - 1) + 1):
                eT = eA if h % 2 == 0 else eB
                pcs[h] = cpool.tile([P, NT], F32, tag="cc", name=f"pcL{h}")
                for jc in range(KC):
                    ctx_mm(h, t, eT, pcs[h], jc)
            for h in pcs:
                ctx_norm(h, t, pcs[h])

        def out_m(m):
            att = opool.tile([P, D], BF16, tag="att")
            for t in range(2):
                ps = mmps.tile([P, NT], F32, tag="mm", name="po")
                for kc in range(KC):
                    nc.tensor.matmul(
                        ps[:],
                        cT[kc][:, m * P : (m + 1) * P],
                        wf_sb[:, kc, t * NT : (t + 1) * NT],
                        start=(kc == 0),
                        stop=(kc == KC - 1),
                    )
                nc.vector.tensor_tensor(
                    att[:, t * NT : (t + 1) * NT],
                    ps[:],
                    bfb[:, t * NT : (t + 1) * NT],
                    AL.add,
                )
            nc.sync.dma_start(out=out_e[1, m * P : (m + 1) * P, :], in_=att[:])
            # LN epilogue
            res = opool.tile([P, D], F32, tag="res")
            ssum = stat.tile([P, 1], F32, tag="ss")
            nc.vector.scalar_tensor_tensor(
                res[:], att[:], 1.0, xn_sb[:, m, :], AL.mult, AL.add, accum_out=ssum[:]
            )
            sq = sqp.tile([P, D], BF16, tag="sq")
            sqs = stat.tile([P, 1], F32, tag="sqs")
            nc.scalar.activation(sq[:], res[:], AF.Square, accum_out=sqs[:])
            mu = stat.tile([P, 1], F32, tag="mu")
            nc.vector.tensor_scalar_mul(mu[:], ssum[:], 1.0 / D)
            msq = stat.tile([P, 1], F32, tag="msq")
            nc.vector.tensor_scalar(msq[:], mu[:], mu[:], None, AL.mult)
            var = stat.tile([P, 1], F32, tag="var")
            nc.vector.tensor_scalar(var[:], sqs[:], 1.0 / D, msq[:], AL.mult, AL.subtract)
            sd = stat.tile([P, 1], F32, tag="sd")
            nc.scalar.activation(sd[:], var[:], AF.Sqrt, bias=epsb[:])
            inv = stat.tile([P, 1], F32, tag="inv")
            nc.vector.reciprocal(inv[:], sd[:])
            # final LN output reuses the att tile (its atted DMA has drained
            # by the time the stats chain finishes)
            if apply_affine:
                nc.vector.tensor_scalar(res[:], res[:], mu[:], inv[:], AL.subtract, AL.mult)
                nc.vector.scalar_tensor_tensor(res[:], res[:], 1.0, gmb[:], AL.mult, AL.mult)
                nc.vector.tensor_tensor(att[:], res[:], btb[:], AL.add)
            else:
                nc.vector.tensor_scalar(att[:], res[:], mu[:], inv[:], AL.subtract, AL.mult)
            nc.sync.dma_start(out=out_e[0, m * P : (m + 1) * P, :], in_=att[:])

        ctx_last(0)
        for m in range(MT // 2):
            out_m(m)
        ctx_last(1)
        for m in range(MT // 2, MT):
            out_m(m)

    _split_excess_waits(nc)
    return nc


def prepare_in_maps(inputs):
    def chunk_pmajor(a):
        # [KC*P, N] -> [P, KC, N] so each partition's row is contiguous HBM
        kcp, n = a.shape
        return np.ascontiguousarray(
            a.reshape(KC, P, n).transpose(1, 0, 2)
        )

    x = np.asarray(inputs["x"], np.float32)
    xr = x.reshape(B, L, DIL, D).transpose(0, 2, 1, 3).reshape(NCORES, L, D)
    shared = {
        "wqT": chunk_pmajor((np.asarray(inputs["Wq"], np.float32).T * SCALE).astype(BF16_NP)),
        "wkT": chunk_pmajor(np.asarray(inputs["Wk"], np.float32).T.astype(BF16_NP)),
        "wvT": chunk_pmajor(np.asarray(inputs["Wv"], np.float32).T.astype(BF16_NP)),
        "wfT": chunk_pmajor(np.asarray(inputs["Wf"], np.float32).T.astype(BF16_NP)),
        "bqc": np.ascontiguousarray(
            (np.asarray(inputs["bq"], np.float32) * SCALE).reshape(MT, P).T
        ),
        "bkc": np.ascontiguousarray(
            np.asarray(inputs["bk"], np.float32).reshape(MT, P).T
        ),
        "bv": np.ascontiguousarray(inputs["bv"], dtype=np.float32),
        "bf": np.ascontiguousarray(inputs["bf"], dtype=np.float32),
        "gam": np.ascontiguousarray(inputs["gamma"], dtype=np.float32),
        "bet": np.ascontiguousarray(inputs["beta"], dtype=np.float32),
    }
    maps = []
    for c in range(NCORES):
        xs = np.ascontiguousarray(xr[c])
        m = dict(shared)
        m["xT"] = chunk_pmajor(np.ascontiguousarray(xs.T).astype(BF16_NP))
        m["xn"] = chunk_pmajor(xs.astype(BF16_NP))
        maps.append(m)
    return maps


def gather_outputs(results):
    outs = np.stack(
        [np.asarray(results[c]["out"]).astype(np.float32) for c in range(NCORES)]
    )
    final = outs[:, 0].reshape(B, DIL, L, D).transpose(0, 2, 1, 3).reshape(B, S, D)
    atted = outs[:, 1].reshape(B, DIL, L, D).transpose(0, 2, 1, 3).reshape(B, S, D)
    return np.ascontiguousarray(final), np.ascontiguousarray(atted)


_GRAPHS = {}


def get_graph(apply_affine=False):
    if apply_affine not in _GRAPHS:
        _GRAPHS[apply_affine] = build_graph(apply_affine)
    return _GRAPHS[apply_affine]


def run(inputs, trace=False, **kw):
    # gamma/beta are fixed to ones/zeros by the reference's setup_inputs;
    # only emit the affine LN ops if they are actually non-identity.
    apply_affine = not (
        np.all(np.asarray(inputs["gamma"]) == 1.0)
        and np.all(np.asarray(inputs["beta"]) == 0.0)
    )
    nc = get_graph(apply_affine)
    maps = prepare_in_maps(inputs)
    res = run_bass_kernel_spmd(nc, maps, core_ids=list(range(NCORES)), trace=trace, **kw)
    return gather_outputs(res.results), res


def kernel(**inputs):
    (final, atted), _ = run(inputs, trace=False)
    return final, atted
